# revision 9
# baseline (speedup 1.0000x reference)
"""Trainium2 Bass kernel for Bahdanau-style attention pooling.

Reference computation (per batch b):
    qp   = q @ Wy + by                         # [B,H]
    h    = tanh(v @ Wx + bx + qp[:,None,:])    # [B,R,H]
    l    = h @ Wa + ba                         # [B,R,1]
    p    = softmax(l, axis=R)                  # [B,R,1]  (ba cancels)
    vw   = sum(p * v, axis=R)                  # [B,Dv]
    returns (vw, p)

Strategy: data-parallel over batch across 8 NeuronCores (8 batches/core).
Single pass over v from HBM per core (64 MiB fp32): v is cast to fp16
during the SWDGE load, xbar-transposed on-chip for the v@Wx contraction
(which needs D on partitions), and the natural-layout fp16 copy is reused
for the probs-weighted sum.  All matmuls fp16 with fp32 PSUM accumulation;
qp/softmax math in fp32.  Softmax skips max-subtraction (logits are O(1)
by construction: tanh(h) in [-1,1], Wa ~ N(0,1/H)).
"""

import numpy as np

import concourse.bacc as bacc
import concourse.bass as bass
import concourse.tile as tile
import concourse.mybir as mybir
from concourse.bass_utils import run_bass_kernel_spmd

F32 = mybir.dt.float32
F16 = mybir.dt.float16
AFT = mybir.ActivationFunctionType

NCORES = 8
B = 64            # total batch
BPC = B // NCORES  # batches per core
R = 2048          # regions
DV = 1024         # v feature dim
DQ = 1024         # q feature dim
H = 512           # hidden
P = 128           # partitions
DC = DV // P      # 8 d-chunks
HC = H // P       # 4 h-chunks
RT = 4            # row tiles per batch (512 rows each)
ROWS = R // RT    # 512
RCH = ROWS // P   # 4 row chunks per row tile


def build_kernel(tc, ctx, reps=1):
    nc = tc.nc
    v_s = nc.dram_tensor("v_s", [BPC, R, DV], F32, kind="ExternalInput").ap()
    q_s = nc.dram_tensor("q_s", [BPC, DQ], F32, kind="ExternalInput").ap()
    Wx = nc.dram_tensor("Wx", [DV, H], F32, kind="ExternalInput").ap()
    bx = nc.dram_tensor("bx", [H], F32, kind="ExternalInput").ap()
    Wy = nc.dram_tensor("Wy", [DQ, H], F32, kind="ExternalInput").ap()
    by = nc.dram_tensor("by", [H], F32, kind="ExternalInput").ap()
    Wa = nc.dram_tensor("Wa", [H, 1], F32, kind="ExternalInput").ap()
    vw_s = nc.dram_tensor("vw_s", [BPC, DV], F32, kind="ExternalOutput").ap()
    probs_s = nc.dram_tensor("probs_s", [BPC, R], F32, kind="ExternalOutput").ap()

    consts = ctx.enter_context(tc.tile_pool(name="consts", bufs=1))

    # ---- constants into SBUF ----
    wx_f16 = consts.tile([P, DC, H], F16, tag="wx")
    nc.gpsimd.dma_start(wx_f16[:], Wx.rearrange("(dc p) h -> p dc h", p=P))
    wa_f16 = consts.tile([P, HC], F16, tag="wa")
    nc.gpsimd.dma_start(wa_f16[:], Wa.rearrange("(hc p) one -> p (hc one)", p=P))
    qT = consts.tile([P, DC, BPC], F32, tag="qT")
    for dc in range(DC):
        nc.sync.dma_start(
            qT[:, dc, :], q_s[:, dc * P:(dc + 1) * P].rearrange("b p -> p b"))
    bxT = consts.tile([P, HC], F32, tag="bxT")
    nc.sync.dma_start(bxT[:], bx.rearrange("(hc p) -> p hc", p=P))
    byT = consts.tile([P, HC], F32, tag="byT")
    nc.sync.dma_start(byT[:], by.rearrange("(hc p) -> p hc", p=P))
    bb = consts.tile([P, HC], F32, tag="bb")
    nc.vector.tensor_add(bb[:], bxT[:], byT[:])

    # biasT[:, hc, b] = (q_b @ Wy)[hc-chunk] + by[hc-chunk] + bx[hc-chunk]
    biasT = consts.tile([P, HC, BPC], F32, tag="biasT")
    with tc.tile_pool(name="qp_psum", bufs=HC, space="PSUM") as qp_pool, \
         tc.tile_pool(name="wy", bufs=1) as wy_pool:
        wy_sb = wy_pool.tile([P, DC, H], F32, tag="wy")
        nc.sync.dma_start(wy_sb[:], Wy.rearrange("(dc p) h -> p dc h", p=P))
        for hc in range(HC):
            qp_ps = qp_pool.tile([P, BPC], F32, tag="qp")
            for dc in range(DC):
                nc.tensor.matmul(
                    qp_ps[:],
                    lhsT=wy_sb[:, dc, hc * P:(hc + 1) * P],
                    rhs=qT[:, dc, :],
                    start=(dc == 0),
                    stop=(dc == DC - 1),
                )
            nc.scalar.add(biasT[:, hc, :], qp_ps[:], bb[:, hc:hc + 1])

    # ---- pools for the main loop ----
    vnat_pool = ctx.enter_context(tc.tile_pool(name="vnat", bufs=8))
    vT_pool = ctx.enter_context(tc.tile_pool(name="vT", bufs=3))
    h_pool = ctx.enter_context(tc.tile_pool(name="h", bufs=2 * HC))
    pre_pool = ctx.enter_context(tc.tile_pool(name="pre", bufs=4, space="PSUM"))
    lpsum_pool = ctx.enter_context(tc.tile_pool(name="lpsum", bufs=2, space="PSUM"))
    wpsum_pool = ctx.enter_context(tc.tile_pool(name="wpsum", bufs=2, space="PSUM"))
    rows_pool = ctx.enter_context(tc.tile_pool(name="rows", bufs=2))
    pdram_pool = ctx.enter_context(tc.tile_pool(name="pdram", bufs=2, space="DRAM"))

    pending_logits = None  # (h_tiles, rt, logits_row)
    pending_wsum = None    # closure

    def flush_logits():
        nonlocal pending_logits
        if pending_logits is None:
            return
        h_tiles, rt, logits_row = pending_logits
        pending_logits = None
        psum_l = lpsum_pool.tile([1, ROWS], F32, tag="lp")
        for hc in range(HC):
            nc.tensor.matmul(
                psum_l[:],
                lhsT=wa_f16[:, hc:hc + 1],
                rhs=h_tiles[hc][:],
                start=(hc == 0),
                stop=(hc == HC - 1),
            )
        nc.vector.tensor_copy(logits_row[:, rt * ROWS:(rt + 1) * ROWS], psum_l[:])

    def make_wsum(b, vnats_b, probs_rs):
        def emit():
            pw = [wpsum_pool.tile([1, H], F32, tag="wp", name=f"wp{b}_{i}")
                  for i in range(2)]
            for c in range(R // P):          # 16 row chunks
                rt_, rc = divmod(c, RCH)
                for dh in range(2):
                    nc.tensor.matmul(
                        pw[dh][:],
                        lhsT=probs_rs[:, c:c + 1],
                        rhs=vnats_b[rt_][:, rc, dh * H:(dh + 1) * H],
                        start=(c == 0),
                        stop=(c == R // P - 1),
                    )
            vw_row = rows_pool.tile([1, DV], F32, tag="vw_row")
            nc.vector.tensor_copy(vw_row[:, :H], pw[0][:])
            nc.vector.tensor_copy(vw_row[:, H:], pw[1][:])
            nc.scalar.dma_start(vw_s[b], vw_row[:])
        return emit

    for b in [b for _ in range(reps) for b in range(BPC)]:
        logits_row = rows_pool.tile([1, R], F32, tag="logits_row")
        vnats_b = []
        for rt in range(RT):
            # load + cast fp32 -> fp16 (SWDGE), natural layout
            vnat = vnat_pool.tile([P, RCH, DV], F16, tag="vnat")
            nc.gpsimd.dma_start(
                vnat[:],
                v_s[b, rt * ROWS:(rt + 1) * ROWS, :].rearrange(
                    "(rc p) d -> p rc d", p=P),
            )
            vnats_b.append(vnat)
            # transpose each [128 rows, 1024 D] -> [128 D, (dc, 128 rows)]
            vT = vT_pool.tile([P, RCH, DC, P], F16, tag="vT")
            for rc in range(RCH):
                nc.sync.dma_start(vT[:, rc], vnat[:, rc, :], transpose=True)
            # pre^T[hc] = sum_dc Wx[dc,hc].T @ vT[dc]  (+bias via ACT)
            h_tiles = []
            for hc in range(HC):
                pre = pre_pool.tile([P, ROWS], F32, tag="pre", space="PSUM")
                for dc in range(DC):
                    nc.tensor.matmul(
                        pre[:],
                        lhsT=wx_f16[:, dc, hc * P:(hc + 1) * P],
                        rhs=vT[:, :, dc, :],
                        start=(dc == 0),
                        stop=(dc == DC - 1),
                    )
                h_t = h_pool.tile([P, ROWS], F16, tag="h")
                nc.scalar.activation(h_t[:], pre[:], AFT.Tanh,
                                     bias=biasT[:, hc, b:b + 1])
                h_tiles.append(h_t)
            if rt == 1 and pending_wsum is not None:
                pending_wsum()
                pending_wsum = None
            flush_logits()
            pending_logits = (h_tiles, rt, logits_row)
        flush_logits()

        # ---- softmax over the full row (fp32, no max subtraction) ----
        exp_row = rows_pool.tile([1, R], F32, tag="exp_row")
        ssum = rows_pool.tile([1, 1], F32, tag="ssum")
        nc.scalar.activation(exp_row[:], logits_row[:], AFT.Exp,
                             accum_out=ssum[:])
        rinv = rows_pool.tile([1, 1], F32, tag="rinv")
        nc.vector.reciprocal(rinv[:], ssum[:])
        probs_row = rows_pool.tile([1, R], F32, tag="probs_row")
        nc.vector.tensor_scalar_mul(probs_row[:], exp_row[:], rinv[:])
        nc.scalar.dma_start(probs_s[b], probs_row[:])
        # reshape [1, 2048] -> [128, 16] via DRAM round trip, cast on DVE
        pd = pdram_pool.tile([R], F32, tag="pd")
        nc.scalar.dma_start(pd[:], probs_row[:])
        prs_f32 = rows_pool.tile([P, R // P], F32, tag="prs_f32")
        nc.scalar.dma_start(prs_f32[:], pd.rearrange("(c p) -> p c", p=P))
        probs_rs = rows_pool.tile([P, R // P], F16, tag="probs_rs")
        nc.vector.tensor_copy(probs_rs[:], prs_f32[:])

        pending_wsum = make_wsum(b, vnats_b, probs_rs)

    pending_wsum()


_NC_CACHE = {}


def _get_nc(reps=1):
    key = ("nc", reps)
    if key not in _NC_CACHE:
        from contextlib import ExitStack
        nc = bacc.Bacc("TRN2", target_bir_lowering=False, debug=False,
                       enable_asserts=False, num_devices=NCORES)
        with tile.TileContext(nc) as tc:
            with ExitStack() as ctx:
                build_kernel(tc, ctx, reps=reps)
        nc.compile()
        _NC_CACHE[key] = nc
    return _NC_CACHE[key]


def kernel(v, q, Wx, bx, Wy, by, Wa, ba):
    nc = _get_nc()
    v = np.ascontiguousarray(np.asarray(v, dtype=np.float32))
    q = np.ascontiguousarray(np.asarray(q, dtype=np.float32))
    Wx = np.ascontiguousarray(np.asarray(Wx, dtype=np.float32))
    bx = np.ascontiguousarray(np.asarray(bx, dtype=np.float32))
    Wy = np.ascontiguousarray(np.asarray(Wy, dtype=np.float32))
    by = np.ascontiguousarray(np.asarray(by, dtype=np.float32))
    Wa = np.ascontiguousarray(np.asarray(Wa, dtype=np.float32))

    in_maps = []
    for c in range(NCORES):
        sl = slice(c * BPC, (c + 1) * BPC)
        in_maps.append({
            "v_s": v[sl], "q_s": q[sl],
            "Wx": Wx, "bx": bx, "Wy": Wy, "by": by, "Wa": Wa,
        })
    res = run_bass_kernel_spmd(nc, in_maps, core_ids=list(range(NCORES)))
    vw = np.concatenate([r["vw_s"] for r in res.results], axis=0)
    probs = np.concatenate([r["probs_s"] for r in res.results], axis=0)
    return vw, probs.reshape(B, R, 1)


# revision 13
# speedup vs baseline: 2944.5043x; 2944.5043x over previous
"""Trainium2 Bass kernel for Bahdanau-style attention pooling.

Reference computation (per batch b):
    qp   = q @ Wy + by                         # [B,H]
    h    = tanh(v @ Wx + bx + qp[:,None,:])    # [B,R,H]
    l    = h @ Wa + ba                         # [B,R,1]
    p    = softmax(l, axis=R)                  # [B,R,1]  (ba cancels)
    vw   = sum(p * v, axis=R)                  # [B,Dv]
    returns (vw, p)

Strategy: data-parallel over batch across 8 NeuronCores (8 batches/core).
Single pass over v from HBM per core (64 MiB fp32): v is cast to fp16
during the SWDGE load, xbar-transposed on-chip for the v@Wx contraction
(which needs D on partitions), and the natural-layout fp16 copy is reused
for the probs-weighted sum.  All matmuls fp16 with fp32 PSUM accumulation;
qp/softmax math in fp32.  Softmax skips max-subtraction (logits are O(1)
by construction: tanh(h) in [-1,1], Wa ~ N(0,1/H)).
"""

import numpy as np

import concourse.bacc as bacc
import concourse.bass as bass
import concourse.tile as tile
import concourse.mybir as mybir
from concourse.bass_utils import run_bass_kernel_spmd

F32 = mybir.dt.float32
F16 = mybir.dt.float16
AFT = mybir.ActivationFunctionType

NCORES = 8
B = 64            # total batch
BPC = B // NCORES  # batches per core
R = 2048          # regions
DV = 1024         # v feature dim
DQ = 1024         # q feature dim
H = 512           # hidden
P = 128           # partitions
DC = DV // P      # 8 d-chunks
HC = H // P       # 4 h-chunks
RT = 4            # row tiles per batch (512 rows each)
ROWS = R // RT    # 512
RCH = ROWS // P   # 4 row chunks per row tile


def build_kernel(tc, ctx, reps=1, loop_n=0):
    nc = tc.nc
    v_s = nc.dram_tensor("v_s", [BPC, R, DV], F32, kind="ExternalInput").ap()
    q_s = nc.dram_tensor("q_s", [BPC, DQ], F32, kind="ExternalInput").ap()
    Wx = nc.dram_tensor("Wx", [DV, H], F32, kind="ExternalInput").ap()
    bx = nc.dram_tensor("bx", [H], F32, kind="ExternalInput").ap()
    Wy = nc.dram_tensor("Wy", [DQ, H], F32, kind="ExternalInput").ap()
    by = nc.dram_tensor("by", [H], F32, kind="ExternalInput").ap()
    Wa = nc.dram_tensor("Wa", [H, 1], F32, kind="ExternalInput").ap()
    vw_s = nc.dram_tensor("vw_s", [BPC, DV], F32, kind="ExternalOutput").ap()
    probs_s = nc.dram_tensor("probs_s", [BPC, R], F32, kind="ExternalOutput").ap()

    consts = ctx.enter_context(tc.tile_pool(name="consts", bufs=1))

    # ---- constants into SBUF ----
    wx_f16 = consts.tile([P, DC, H], F16, tag="wx")
    nc.gpsimd.dma_start(wx_f16[:], Wx.rearrange("(dc p) h -> p dc h", p=P))
    wa_f16 = consts.tile([P, HC], F16, tag="wa")
    nc.gpsimd.dma_start(wa_f16[:], Wa.rearrange("(hc p) one -> p (hc one)", p=P))
    qT = consts.tile([P, DC, BPC], F32, tag="qT")
    for dc in range(DC):
        nc.sync.dma_start(
            qT[:, dc, :], q_s[:, dc * P:(dc + 1) * P].rearrange("b p -> p b"))
    bxT = consts.tile([P, HC], F32, tag="bxT")
    nc.sync.dma_start(bxT[:], bx.rearrange("(hc p) -> p hc", p=P))
    byT = consts.tile([P, HC], F32, tag="byT")
    nc.sync.dma_start(byT[:], by.rearrange("(hc p) -> p hc", p=P))
    bb = consts.tile([P, HC], F32, tag="bb")
    nc.vector.tensor_add(bb[:], bxT[:], byT[:])

    # biasT[:, hc, b] = (q_b @ Wy)[hc-chunk] + by[hc-chunk] + bx[hc-chunk]
    biasT = consts.tile([P, HC, BPC], F32, tag="biasT")
    with tc.tile_pool(name="qp_psum", bufs=HC, space="PSUM") as qp_pool, \
         tc.tile_pool(name="wy", bufs=1) as wy_pool:
        wy_sb = wy_pool.tile([P, DC, H], F32, tag="wy")
        nc.sync.dma_start(wy_sb[:], Wy.rearrange("(dc p) h -> p dc h", p=P))
        for hc in range(HC):
            qp_ps = qp_pool.tile([P, BPC], F32, tag="qp")
            for dc in range(DC):
                nc.tensor.matmul(
                    qp_ps[:],
                    lhsT=wy_sb[:, dc, hc * P:(hc + 1) * P],
                    rhs=qT[:, dc, :],
                    start=(dc == 0),
                    stop=(dc == DC - 1),
                )
            nc.scalar.add(biasT[:, hc, :], qp_ps[:], bb[:, hc:hc + 1])

    # ---- pools for the main loop ----
    vnat_pool = ctx.enter_context(tc.tile_pool(name="vnat", bufs=8))
    vT_pool = ctx.enter_context(tc.tile_pool(name="vT", bufs=3))
    h_pool = ctx.enter_context(tc.tile_pool(name="h", bufs=2 * HC))
    pre_pool = ctx.enter_context(tc.tile_pool(name="pre", bufs=4, space="PSUM"))
    lpsum_pool = ctx.enter_context(tc.tile_pool(name="lpsum", bufs=2, space="PSUM"))
    wpsum_pool = ctx.enter_context(tc.tile_pool(name="wpsum", bufs=2, space="PSUM"))
    rows_pool = ctx.enter_context(tc.tile_pool(name="rows", bufs=2))
    pdram_pool = ctx.enter_context(tc.tile_pool(name="pdram", bufs=2, space="DRAM"))

    pending_logits = None  # (h_tiles, rt, logits_row)
    pending_wsum = None    # closure

    def flush_logits():
        nonlocal pending_logits
        if pending_logits is None:
            return
        h_tiles, rt, logits_row = pending_logits
        pending_logits = None
        psum_l = lpsum_pool.tile([1, ROWS], F32, tag="lp")
        for hc in range(HC):
            nc.tensor.matmul(
                psum_l[:],
                lhsT=wa_f16[:, hc:hc + 1],
                rhs=h_tiles[hc][:],
                start=(hc == 0),
                stop=(hc == HC - 1),
            )
        nc.vector.tensor_copy(logits_row[:, rt * ROWS:(rt + 1) * ROWS], psum_l[:])

    def make_wsum(b, vnats_b, probs_rs):
        def emit():
            pw = [wpsum_pool.tile([1, H], F32, tag="wp", name=f"wp{b}_{i}")
                  for i in range(2)]
            for c in range(R // P):          # 16 row chunks
                rt_, rc = divmod(c, RCH)
                for dh in range(2):
                    nc.tensor.matmul(
                        pw[dh][:],
                        lhsT=probs_rs[:, c:c + 1],
                        rhs=vnats_b[rt_][:, rc, dh * H:(dh + 1) * H],
                        start=(c == 0),
                        stop=(c == R // P - 1),
                    )
            vw_row = rows_pool.tile([1, DV], F32, tag="vw_row")
            nc.vector.tensor_copy(vw_row[:, :H], pw[0][:])
            nc.vector.tensor_copy(vw_row[:, H:], pw[1][:])
            nc.scalar.dma_start(vw_s[b], vw_row[:])
        return emit

    def batch_body(b):
        nonlocal pending_logits, pending_wsum
        logits_row = rows_pool.tile([1, R], F32, tag="logits_row",
                                    name=f"lr{b}")
        vnats_b = []
        for rt in range(RT):
            # load + cast fp32 -> fp16 (SWDGE), natural layout
            vnat = vnat_pool.tile([P, RCH, DV], F16, tag="vnat")
            nc.gpsimd.dma_start(
                vnat[:],
                v_s[b, rt * ROWS:(rt + 1) * ROWS, :].rearrange(
                    "(rc p) d -> p rc d", p=P),
            )
            vnats_b.append(vnat)
            # transpose each [128 rows, 1024 D] -> [128 D, (dc, 128 rows)]
            vT = vT_pool.tile([P, RCH, DC, P], F16, tag="vT")
            for rc in range(RCH):
                nc.sync.dma_start(vT[:, rc], vnat[:, rc, :], transpose=True)
            # pre^T[hc] = sum_dc Wx[dc,hc].T @ vT[dc]  (+bias via ACT)
            h_tiles = []
            for hc in range(HC):
                pre = pre_pool.tile([P, ROWS], F32, tag="pre", space="PSUM")
                for dc in range(DC):
                    nc.tensor.matmul(
                        pre[:],
                        lhsT=wx_f16[:, dc, hc * P:(hc + 1) * P],
                        rhs=vT[:, :, dc, :],
                        start=(dc == 0),
                        stop=(dc == DC - 1),
                    )
                h_t = h_pool.tile([P, ROWS], F16, tag="h")
                nc.scalar.activation(h_t[:], pre[:], AFT.Tanh,
                                     bias=biasT[:, hc, b:b + 1])
                h_tiles.append(h_t)
            if rt == 1 and pending_wsum is not None:
                pending_wsum()
                pending_wsum = None
            flush_logits()
            pending_logits = (h_tiles, rt, logits_row)
        flush_logits()

        # ---- softmax over the full row (fp32, no max subtraction) ----
        exp_row = rows_pool.tile([1, R], F32, tag="exp_row")
        ssum = rows_pool.tile([1, 1], F32, tag="ssum")
        nc.scalar.activation(exp_row[:], logits_row[:], AFT.Exp,
                             accum_out=ssum[:])
        rinv = rows_pool.tile([1, 1], F32, tag="rinv")
        nc.vector.reciprocal(rinv[:], ssum[:])
        probs_row = rows_pool.tile([1, R], F32, tag="probs_row")
        nc.vector.tensor_scalar_mul(probs_row[:], exp_row[:], rinv[:])
        nc.scalar.dma_start(probs_s[b], probs_row[:])
        # reshape [1, 2048] -> [128, 16] via DRAM round trip, cast on DVE
        pd = pdram_pool.tile([R], F32, tag="pd")
        nc.scalar.dma_start(pd[:], probs_row[:])
        prs_f32 = rows_pool.tile([P, R // P], F32, tag="prs_f32")
        nc.scalar.dma_start(prs_f32[:], pd.rearrange("(c p) -> p c", p=P))
        probs_rs = rows_pool.tile([P, R // P], F16, tag="probs_rs")
        nc.vector.tensor_copy(probs_rs[:], prs_f32[:])

        pending_wsum = make_wsum(b, vnats_b, probs_rs)

    def all_batches():
        nonlocal pending_wsum
        for b in range(BPC):
            batch_body(b)
        pending_wsum()
        pending_wsum = None

    if loop_n:
        with tc.For_i(0, loop_n, 1):
            all_batches()
    else:
        for _ in range(reps):
            all_batches()


_NC_CACHE = {}


def _get_nc(reps=1, loop_n=0):
    key = ("nc", reps, loop_n)
    if key not in _NC_CACHE:
        from contextlib import ExitStack
        nc = bacc.Bacc("TRN2", target_bir_lowering=False, debug=False,
                       enable_asserts=False, num_devices=NCORES)
        with tile.TileContext(nc) as tc:
            with ExitStack() as ctx:
                build_kernel(tc, ctx, reps=reps, loop_n=loop_n)
        nc.compile()
        _NC_CACHE[key] = nc
    return _NC_CACHE[key]


def kernel(v, q, Wx, bx, Wy, by, Wa, ba):
    nc = _get_nc()
    v = np.ascontiguousarray(np.asarray(v, dtype=np.float32))
    q = np.ascontiguousarray(np.asarray(q, dtype=np.float32))
    Wx = np.ascontiguousarray(np.asarray(Wx, dtype=np.float32))
    bx = np.ascontiguousarray(np.asarray(bx, dtype=np.float32))
    Wy = np.ascontiguousarray(np.asarray(Wy, dtype=np.float32))
    by = np.ascontiguousarray(np.asarray(by, dtype=np.float32))
    Wa = np.ascontiguousarray(np.asarray(Wa, dtype=np.float32))

    in_maps = []
    for c in range(NCORES):
        sl = slice(c * BPC, (c + 1) * BPC)
        in_maps.append({
            "v_s": v[sl], "q_s": q[sl],
            "Wx": Wx, "bx": bx, "Wy": Wy, "by": by, "Wa": Wa,
        })
    res = run_bass_kernel_spmd(nc, in_maps, core_ids=list(range(NCORES)))
    vw = np.concatenate([r["vw_s"] for r in res.results], axis=0)
    probs = np.concatenate([r["probs_s"] for r in res.results], axis=0)
    return vw, probs.reshape(B, R, 1)
